# revision 5
# baseline (speedup 1.0000x reference)
"""Trainium2 Bass kernel for attention-weighted pooling.

Computes, for x[B,T,D], W[D,1], b[T,1]:
    et = tanh(x @ W + b)            # (B, T)
    at = softmax(et, axis=-1)       # (B, T)
    out = einsum('btd,bt->bd', x, at)

Sharding: pure data parallel over batch across 8 NeuronCores (4 batches per
core); W and b replicated. No collectives.

Key structure (per core, streaming single pass over x):
  - tanh output is bounded in [-1, 1], so softmax needs no max subtraction;
    exp() cannot overflow. Normalization by the denominator is deferred to
    the very end, so x is read from HBM exactly once (memory roofline).
  - x is cast fp32 -> fp16 during the DMA itself (SWDGE cast): HBM traffic
    stays the required 32 MiB/core of fp32, but on-chip x is half the bytes.
  - ALL 16 x chunks (1 MiB fp16 each) stay resident in SBUF so the DMA
    stream is never throttled by buffer reuse.
  - Per 2-MiB chunk [128 x 8 x 512]: ONE whole-chunk DVE tensor_mul
    (fp16 2x mode) + ONE whole-chunk reduce_sum ([P,8,512] -> [P,8,1])
    replace the per-subtile fused dot ops; ACT only does tanh + exp.
  - b is PRE-PERMUTED ON HOST to match the on-chip t-layout, so its DMA is
    one clean [128, 32] HWDGE transfer instead of 2048 8-byte descriptors.
  - Denominator S for every batch via free-dim reduce + ones-matmul on PE
    (no gpsimd custom op -> no mid-stream LOAD_LIB bandwidth dip).
  - Pair-row DMA layout: each descriptor covers 2 adjacent t-rows (4 KiB
    contiguous HBM). Softmax/pooling are permutation-invariant over t.
"""

import sys

sys.path.insert(0, "/opt/trn_rl_repo")

import numpy as np

B, T, D = 32, 4096, 512
N_CORES = 8
B_LOCAL = B // N_CORES          # 4 batches per core
P = 128                         # SBUF partitions
TS_T = 1024                     # t-rows per super-tile (2 MiB fp32 DMA)
N_ST = T // TS_T                # 4 super-tiles per batch
N_J = TS_T // P                 # 8 t-subtiles per super-tile
N_STT = 4                       # subtiles on the fused-DVE path; rest on ACT

# 8-rows-per-partition layout: within a chunk of nj*128 rows starting at t0,
# partition p holds rows t0 + nj*p + j (j = 0..nj-1), so each DMA descriptor
# covers nj*2 KiB contiguous HBM (16 KiB for full chunks -> 4x fewer
# descriptors -> 4x less SWDGE descriptor-ring fetch traffic, which is what
# made SDMA engine 15 a straggler). Softmax/pooling are permutation-invariant
# over t; only the host-side b permutation must match this map.
#
# Batches 0..B_LOCAL-2 stream full 2-MiB chunks; the LAST batch tapers its
# final chunk so the post-stream compute drain is short (the taper's extra
# per-chunk overhead lands after the stream has already ended).
PLAN_FULL = [(st * TS_T, N_J) for st in range(N_ST)]
PLAN_TAIL = [(0, 8), (1024, 8), (2048, 8), (3072, 6), (3840, 2)]

_PROGRAM = None


def _build_program():
    import concourse.bacc as bacc
    import concourse.mybir as mybir
    import concourse.tile as tile

    f32 = mybir.dt.float32
    f16 = mybir.dt.float16
    nc = bacc.Bacc("TRN2", target_bir_lowering=False, debug=False)

    x_d = nc.dram_tensor("x", [B_LOCAL, T, D], f32, kind="ExternalInput")
    W_d = nc.dram_tensor("W", [D, 1], f32, kind="ExternalInput")
    b_d = nc.dram_tensor("b", [P, 2 * N_ST * N_J], f32, kind="ExternalInput")
    o_d = nc.dram_tensor("out", [B_LOCAL, D], f32, kind="ExternalOutput")

    with tile.TileContext(nc) as tc:
        with (
            tc.tile_pool(name="consts", bufs=1) as consts,
            # All 16 x chunks stay resident so the DMA stream is never
            # throttled by SBUF slot reuse.
            tc.tile_pool(name="xin", bufs=16) as xin,
            tc.tile_pool(name="prod", bufs=6) as prod_pool,
            tc.tile_pool(name="small", bufs=8) as small,
            tc.tile_pool(name="pbuf", bufs=2) as pbuf_pool,
            tc.tile_pool(name="acc_psum", bufs=2, space="PSUM") as acc_psum_pool,
            tc.tile_pool(name="s_psum", bufs=2, space="PSUM") as s_psum_pool,
        ):
            plans = [PLAN_FULL] * (B_LOCAL - 1) + [PLAN_TAIL]
            xbufs = {8: 15, 6: 1, 2: 1}

            def issue_x_dma(bb, t0, nj):
                # One nj*2 KiB descriptor per partition per chunk.
                xt = xin.tile(
                    [P, nj, D], f16, tag=f"xt{nj}", bufs=xbufs[nj], name="xt"
                )
                nc.gpsimd.dma_start(
                    xt[:],
                    x_d.ap()[bb, t0 : t0 + nj * P, :].rearrange(
                        "(p r) d -> p r d", p=P
                    ),
                )
                return xt

            # Pre-issue the first x chunk so the SWDGE queue starts streaming
            # x before the W broadcast.
            pre = {(0, 0): issue_x_dma(0, 0, N_J)}

            # W broadcast to all 128 partitions, cast to fp16: [128, D].
            w_bcast = consts.tile([P, D], f16)
            nc.gpsimd.dma_start(
                w_bcast[:],
                W_d.ap().rearrange("d one -> one d").broadcast_to([P, D]),
            )
            # b arrives host-pre-permuted to the on-chip layout(s): section 0
            # for the uniform plan, section 1 for the tapered last batch.
            b_buf = consts.tile([P, 2 * N_ST * N_J], f32)
            nc.sync.dma_start(b_buf[:], b_d.ap())
            ones_col = consts.tile([P, 1], f32)
            nc.vector.memset(ones_col[:], 1.0)
            # All batch outputs gathered on one partition; single 8 KiB
            # contiguous DMA at the end.
            out_all = consts.tile([1, B_LOCAL * D], f32)

            for bb in range(B_LOCAL):
                p_buf = pbuf_pool.tile([P, T // P], f16)
                acc = acc_psum_pool.tile([1, D], f32)

                chunks = plans[bb]
                sel = 1 if bb == B_LOCAL - 1 else 0
                total_mm = N_ST * N_J
                mm_idx = 0
                cum = 0
                for ci, (t0, nj) in enumerate(chunks):
                    col0 = cum
                    xt = pre.pop((bb, ci), None)
                    if xt is None:
                        xt = issue_x_dma(bb, t0, nj)
                    # Half the subtiles: fused mult+reduce on DVE
                    # (scalar_tensor_tensor, 1x uop ~690ns). Other half: plain
                    # tensor_tensor mult on DVE (fp16 2x mode ~365ns) with the
                    # reduce offloaded to ACT (activation Copy + accum_out).
                    # This splits the dot-product work so both engines stay
                    # under the DMA pace.
                    # Tiny tail chunks take the all-fused DVE path: one DVE
                    # op has much lower serial latency than mult->ACT->accum.
                    n_stt = nj if nj <= 2 else nj // 2
                    elin = small.tile([P, nj], f32, tag=f"elin{nj}")
                    for j in range(n_stt):
                        scratch = prod_pool.tile([P, D], f16, tag="scr")
                        nc.vector.scalar_tensor_tensor(
                            out=scratch[:],
                            in0=xt[:, j, :],
                            scalar=1.0,
                            in1=w_bcast[:],
                            op0=mybir.AluOpType.mult,
                            op1=mybir.AluOpType.mult,
                            accum_out=elin[:, j : j + 1],
                        )
                    for j in range(n_stt, nj):
                        prod = prod_pool.tile([P, D], f16, tag="prod")
                        nc.vector.tensor_mul(prod[:], xt[:, j, :], w_bcast[:])
                        nc.scalar.activation(
                            prod[:],
                            prod[:],
                            mybir.ActivationFunctionType.Copy,
                            accum_out=elin[:, j : j + 1],
                        )
                    ee = small.tile([P, nj], f32, tag=f"ee{nj}")
                    nc.vector.tensor_add(
                        ee[:],
                        elin[:],
                        b_buf[:, sel * 32 + cum : sel * 32 + cum + nj],
                    )
                    et = small.tile([P, nj], f32, tag=f"et{nj}")
                    nc.scalar.activation(
                        et[:], ee[:], mybir.ActivationFunctionType.Tanh
                    )
                    nc.scalar.activation(
                        p_buf[:, col0 : col0 + nj],
                        et[:],
                        mybir.ActivationFunctionType.Exp,
                    )
                    cum += nj
                    for j in range(nj):
                        nc.tensor.matmul(
                            acc[:],
                            p_buf[:, col0 + j : col0 + j + 1],
                            xt[:, j, :],
                            start=(mm_idx == 0),
                            stop=(mm_idx == total_mm - 1),
                        )
                        mm_idx += 1

                # denominator S = sum_t p_t  (free-dim reduce, then
                # cross-partition reduce via ones-matmul on PE; the tiny S
                # matmul queues right behind the batch's pooling matmuls)
                ssum = small.tile([P, 1], f32)
                nc.vector.reduce_sum(ssum[:], p_buf[:], axis=mybir.AxisListType.X)
                s_ps = s_psum_pool.tile([1, 1], f32)
                nc.tensor.matmul(s_ps[:], ssum[:], ones_col[:])
                sinv = small.tile([1, 1], f32)
                nc.vector.reciprocal(sinv[:], s_ps[:])
                if bb < B_LOCAL - 1:
                    # Mid-stream: final scale on ACT (it is idle), overlapping
                    # the ongoing stream.
                    nc.scalar.mul(
                        out_all[:, bb * D : (bb + 1) * D], acc[:], sinv[:]
                    )
                else:
                    # Last batch: scale on DVE right after its own reciprocal
                    # (no cross-engine hop on the exposed tail).
                    nc.vector.tensor_scalar_mul(
                        out_all[:, bb * D : (bb + 1) * D], acc[:], sinv[:]
                    )

            nc.sync.dma_start(
                o_d.ap().rearrange("(one b) d -> one (b d)", one=1), out_all[:]
            )

    nc.compile()
    return nc


def _get_program():
    global _PROGRAM
    if _PROGRAM is None:
        _PROGRAM = _build_program()
    return _PROGRAM


def _permute_b(b):
    """Host-side: lay b out as [P, 64] matching both on-chip layouts
    (cols 0..31: uniform plan; cols 32..63: tapered last-batch plan)."""
    bv = b[:, 0]
    out = np.zeros((P, 2 * N_ST * N_J), dtype=np.float32)
    p = np.arange(P)
    for sec, plan in ((0, PLAN_FULL), (1, PLAN_TAIL)):
        cum = 0
        for t0, nj in plan:
            for j in range(nj):
                out[:, sec * 32 + cum + j] = bv[t0 + nj * p + j]
            cum += nj
    return np.ascontiguousarray(out)


def _shard_inputs(x, W, b):
    x = np.ascontiguousarray(np.asarray(x, dtype=np.float32))
    W = np.ascontiguousarray(np.asarray(W, dtype=np.float32))
    b = np.ascontiguousarray(np.asarray(b, dtype=np.float32))
    b_perm = _permute_b(b)
    return [
        {"x": x[c * B_LOCAL : (c + 1) * B_LOCAL], "W": W, "b": b_perm}
        for c in range(N_CORES)
    ]


def _install_ntff_hook_shim():
    """The agent image's ``antenv`` lacks ``axon_hooks``, so the boot-time
    NTFF hook registration silently degrades. Recreate the module in
    sys.modules and register the ctypes hook against libaxon_pjrt.so."""
    import types

    if "antenv.axon_hooks" in sys.modules:
        return
    mod = types.ModuleType("antenv.axon_hooks")
    _hook = [None]
    mod.set_axon_ntff_profile_hook = lambda h: _hook.__setitem__(0, h)
    mod.get_axon_ntff_profile_hook = lambda: _hook[0]
    import antenv

    antenv.axon_hooks = mod
    sys.modules["antenv.axon_hooks"] = mod
    try:
        sys.path.insert(0, "/root/.axon_site")
        from trn_agent_boot.trn_boot import _ntff_profile_via_ctypes

        mod.set_axon_ntff_profile_hook(
            _ntff_profile_via_ctypes("/opt/axon/libaxon_pjrt.so")
        )
    except Exception as e:  # profiling is best-effort; run still works
        print(f"NTFF hook shim failed ({e}); tracing disabled", file=sys.stderr)


def _run(in_maps, trace=False):
    from concourse.bass_utils import run_bass_kernel_spmd

    nc = _get_program()
    kwargs = {}
    if trace:
        _install_ntff_hook_shim()
        kwargs = {"trace": True, "trace_cores": [0]}
    return run_bass_kernel_spmd(nc, in_maps, core_ids=list(range(N_CORES)), **kwargs)


def kernel(x, W, b):
    res = _run(_shard_inputs(x, W, b))
    return np.concatenate(
        [res.results[c]["out"] for c in range(N_CORES)], axis=0
    ).astype(np.float32)


def kernel_profiled(x, W, b):
    """Like kernel() but also returns the NTFF-measured exec time in ns."""
    res = _run(_shard_inputs(x, W, b), trace=True)
    out = np.concatenate(
        [res.results[c]["out"] for c in range(N_CORES)], axis=0
    ).astype(np.float32)
    return out, res


# revision 7
# speedup vs baseline: 1.0350x; 1.0350x over previous
"""Trainium2 Bass kernel for attention-weighted pooling.

Computes, for x[B,T,D], W[D,1], b[T,1]:
    et = tanh(x @ W + b)            # (B, T)
    at = softmax(et, axis=-1)       # (B, T)
    out = einsum('btd,bt->bd', x, at)

Sharding: pure data parallel over batch across 8 NeuronCores (4 batches per
core); W and b replicated. No collectives.

Key structure (per core, streaming single pass over x):
  - tanh output is bounded in [-1, 1], so softmax needs no max subtraction;
    exp() cannot overflow. Normalization by the denominator is deferred to
    the very end, so x is read from HBM exactly once (memory roofline).
  - x is cast fp32 -> fp16 during the DMA itself (SWDGE cast): HBM traffic
    stays the required 32 MiB/core of fp32, but on-chip x is half the bytes.
  - ALL 16 x chunks (1 MiB fp16 each) stay resident in SBUF so the DMA
    stream is never throttled by buffer reuse.
  - Per 2-MiB chunk [128 x 8 x 512]: ONE whole-chunk DVE tensor_mul
    (fp16 2x mode) + ONE whole-chunk reduce_sum ([P,8,512] -> [P,8,1])
    replace the per-subtile fused dot ops; ACT only does tanh + exp.
  - b is PRE-PERMUTED ON HOST to match the on-chip t-layout, so its DMA is
    one clean [128, 32] HWDGE transfer instead of 2048 8-byte descriptors.
  - Denominator S for every batch via free-dim reduce + ones-matmul on PE
    (no gpsimd custom op -> no mid-stream LOAD_LIB bandwidth dip).
  - Pair-row DMA layout: each descriptor covers 2 adjacent t-rows (4 KiB
    contiguous HBM). Softmax/pooling are permutation-invariant over t.
"""

import sys

sys.path.insert(0, "/opt/trn_rl_repo")

import numpy as np

B, T, D = 32, 4096, 512
N_CORES = 8
B_LOCAL = B // N_CORES          # 4 batches per core
P = 128                         # SBUF partitions
TS_T = 1024                     # t-rows per super-tile (2 MiB fp32 DMA)
N_ST = T // TS_T                # 4 super-tiles per batch
N_J = TS_T // P                 # 8 t-subtiles per super-tile
N_STT = 4                       # subtiles on the fused-DVE path; rest on ACT

# 8-rows-per-partition layout: within a chunk of nj*128 rows starting at t0,
# partition p holds rows t0 + nj*p + j (j = 0..nj-1), so each DMA descriptor
# covers nj*2 KiB contiguous HBM (16 KiB for full chunks -> 4x fewer
# descriptors -> 4x less SWDGE descriptor-ring fetch traffic, which is what
# made SDMA engine 15 a straggler). Softmax/pooling are permutation-invariant
# over t; only the host-side b permutation must match this map.
#
# Batches 0..B_LOCAL-2 stream full 2-MiB chunks; the LAST batch tapers its
# final chunk so the post-stream compute drain is short (the taper's extra
# per-chunk overhead lands after the stream has already ended).
PLAN_FULL = [(st * TS_T, N_J) for st in range(N_ST)]
PLAN_TAIL = [(0, 8), (1024, 8), (2048, 8), (3072, 7), (3968, 1)]

_PROGRAM = None


def _build_program():
    import concourse.bacc as bacc
    import concourse.mybir as mybir
    import concourse.tile as tile

    f32 = mybir.dt.float32
    f16 = mybir.dt.float16
    nc = bacc.Bacc("TRN2", target_bir_lowering=False, debug=False)

    x_d = nc.dram_tensor("x", [B_LOCAL, T, D], f32, kind="ExternalInput")
    W_d = nc.dram_tensor("W", [D, 1], f32, kind="ExternalInput")
    b_d = nc.dram_tensor("b", [P, 2 * N_ST * N_J], f32, kind="ExternalInput")
    o_d = nc.dram_tensor("out", [B_LOCAL, D], f32, kind="ExternalOutput")

    with tile.TileContext(nc) as tc:
        with (
            tc.tile_pool(name="consts", bufs=1) as consts,
            # All 16 x chunks stay resident so the DMA stream is never
            # throttled by SBUF slot reuse.
            tc.tile_pool(name="xin", bufs=16) as xin,
            tc.tile_pool(name="prod", bufs=6) as prod_pool,
            tc.tile_pool(name="small", bufs=8) as small,
            tc.tile_pool(name="pbuf", bufs=2) as pbuf_pool,
            tc.tile_pool(name="acc_psum", bufs=2, space="PSUM") as acc_psum_pool,
            tc.tile_pool(name="s_psum", bufs=2, space="PSUM") as s_psum_pool,
        ):
            plans = [PLAN_FULL] * (B_LOCAL - 1) + [PLAN_TAIL]
            xbufs = {8: 15, 7: 1, 1: 1}

            def issue_x_dma(bb, t0, nj):
                # One nj*2 KiB descriptor per partition per chunk.
                xt = xin.tile(
                    [P, nj, D], f16, tag=f"xt{nj}", bufs=xbufs[nj], name="xt"
                )
                nc.gpsimd.dma_start(
                    xt[:],
                    x_d.ap()[bb, t0 : t0 + nj * P, :].rearrange(
                        "(p r) d -> p r d", p=P
                    ),
                )
                return xt

            # Pre-issue the first x chunk so the SWDGE queue starts streaming
            # x before the W broadcast.
            pre = {(0, 0): issue_x_dma(0, 0, N_J)}

            # W broadcast to all 128 partitions, cast to fp16: [128, D].
            w_bcast = consts.tile([P, D], f16)
            nc.gpsimd.dma_start(
                w_bcast[:],
                W_d.ap().rearrange("d one -> one d").broadcast_to([P, D]),
            )
            # b arrives host-pre-permuted to the on-chip layout(s): section 0
            # for the uniform plan, section 1 for the tapered last batch.
            b_buf = consts.tile([P, 2 * N_ST * N_J], f32)
            nc.sync.dma_start(b_buf[:], b_d.ap())
            ones_col = consts.tile([P, 1], f32)
            nc.vector.memset(ones_col[:], 1.0)
            # W replicated 4x along the free dim for the merged multiply.
            w_rep = consts.tile([P, 4 * D], f16)
            for j in range(4):
                nc.vector.tensor_copy(w_rep[:, j * D : (j + 1) * D], w_bcast[:])
            # All batch outputs gathered on one partition; single 8 KiB
            # contiguous DMA at the end.
            out_all = consts.tile([1, B_LOCAL * D], f32)

            for bb in range(B_LOCAL):
                p_buf = pbuf_pool.tile([P, T // P], f16)
                acc = acc_psum_pool.tile([1, D], f32)

                chunks = plans[bb]
                sel = 1 if bb == B_LOCAL - 1 else 0
                total_mm = N_ST * N_J
                mm_idx = 0
                cum = 0
                for ci, (t0, nj) in enumerate(chunks):
                    col0 = cum
                    xt = pre.pop((bb, ci), None)
                    if xt is None:
                        xt = issue_x_dma(bb, t0, nj)
                    # Half the subtiles: fused mult+reduce on DVE
                    # (scalar_tensor_tensor, 1x uop ~690ns). Other half: plain
                    # tensor_tensor mult on DVE (fp16 2x mode ~365ns) with the
                    # reduce offloaded to ACT (activation Copy + accum_out).
                    # This splits the dot-product work so both engines stay
                    # under the DMA pace.
                    # Tiny tail chunks take the all-fused DVE path: one DVE
                    # op has much lower serial latency than mult->ACT->accum.
                    n_stt = nj if nj <= 2 else (nj + 1) // 2
                    n_act = nj - n_stt
                    elin = small.tile([P, nj], f32, tag=f"elin{nj}")
                    if n_act:
                        # ONE merged tensor_mul for all ACT-path subtiles
                        # (fp16 2x mode; one instruction instead of n_act).
                        prod = prod_pool.tile([P, n_act, D], f16, tag=f"prod{n_act}")
                        nc.vector.tensor_mul(
                            prod[:].rearrange("p j d -> p (j d)"),
                            xt[:, n_stt:nj, :].rearrange("p j d -> p (j d)"),
                            w_rep[:, : n_act * D],
                        )
                    for j in range(n_stt):
                        scratch = prod_pool.tile([P, D], f16, tag="scr")
                        nc.vector.scalar_tensor_tensor(
                            out=scratch[:],
                            in0=xt[:, j, :],
                            scalar=1.0,
                            in1=w_bcast[:],
                            op0=mybir.AluOpType.mult,
                            op1=mybir.AluOpType.mult,
                            accum_out=elin[:, j : j + 1],
                        )
                    for j in range(n_stt, nj):
                        nc.scalar.activation(
                            prod[:, j - n_stt, :],
                            prod[:, j - n_stt, :],
                            mybir.ActivationFunctionType.Copy,
                            accum_out=elin[:, j : j + 1],
                        )
                    ee = small.tile([P, nj], f32, tag=f"ee{nj}")
                    nc.vector.tensor_add(
                        ee[:],
                        elin[:],
                        b_buf[:, sel * 32 + cum : sel * 32 + cum + nj],
                    )
                    et = small.tile([P, nj], f32, tag=f"et{nj}")
                    nc.scalar.activation(
                        et[:], ee[:], mybir.ActivationFunctionType.Tanh
                    )
                    nc.scalar.activation(
                        p_buf[:, col0 : col0 + nj],
                        et[:],
                        mybir.ActivationFunctionType.Exp,
                    )
                    cum += nj
                    for j in range(nj):
                        nc.tensor.matmul(
                            acc[:],
                            p_buf[:, col0 + j : col0 + j + 1],
                            xt[:, j, :],
                            start=(mm_idx == 0),
                            stop=(mm_idx == total_mm - 1),
                        )
                        mm_idx += 1

                # denominator S = sum_t p_t  (free-dim reduce, then
                # cross-partition reduce via ones-matmul on PE; the tiny S
                # matmul queues right behind the batch's pooling matmuls)
                ssum = small.tile([P, 1], f32)
                nc.vector.reduce_sum(ssum[:], p_buf[:], axis=mybir.AxisListType.X)
                s_ps = s_psum_pool.tile([1, 1], f32)
                nc.tensor.matmul(s_ps[:], ssum[:], ones_col[:])
                sinv = small.tile([1, 1], f32)
                nc.vector.reciprocal(sinv[:], s_ps[:])
                if bb < B_LOCAL - 1:
                    # Mid-stream: final scale on ACT (it is idle), overlapping
                    # the ongoing stream.
                    nc.scalar.mul(
                        out_all[:, bb * D : (bb + 1) * D], acc[:], sinv[:]
                    )
                else:
                    # Last batch: scale on DVE right after its own reciprocal
                    # (no cross-engine hop on the exposed tail).
                    nc.vector.tensor_scalar_mul(
                        out_all[:, bb * D : (bb + 1) * D], acc[:], sinv[:]
                    )

            nc.sync.dma_start(
                o_d.ap().rearrange("(one b) d -> one (b d)", one=1), out_all[:]
            )

    nc.compile()
    return nc


def _get_program():
    global _PROGRAM
    if _PROGRAM is None:
        _PROGRAM = _build_program()
    return _PROGRAM


def _permute_b(b):
    """Host-side: lay b out as [P, 64] matching both on-chip layouts
    (cols 0..31: uniform plan; cols 32..63: tapered last-batch plan)."""
    bv = b[:, 0]
    out = np.zeros((P, 2 * N_ST * N_J), dtype=np.float32)
    p = np.arange(P)
    for sec, plan in ((0, PLAN_FULL), (1, PLAN_TAIL)):
        cum = 0
        for t0, nj in plan:
            for j in range(nj):
                out[:, sec * 32 + cum + j] = bv[t0 + nj * p + j]
            cum += nj
    return np.ascontiguousarray(out)


def _shard_inputs(x, W, b):
    x = np.ascontiguousarray(np.asarray(x, dtype=np.float32))
    W = np.ascontiguousarray(np.asarray(W, dtype=np.float32))
    b = np.ascontiguousarray(np.asarray(b, dtype=np.float32))
    b_perm = _permute_b(b)
    return [
        {"x": x[c * B_LOCAL : (c + 1) * B_LOCAL], "W": W, "b": b_perm}
        for c in range(N_CORES)
    ]


def _install_ntff_hook_shim():
    """The agent image's ``antenv`` lacks ``axon_hooks``, so the boot-time
    NTFF hook registration silently degrades. Recreate the module in
    sys.modules and register the ctypes hook against libaxon_pjrt.so."""
    import types

    if "antenv.axon_hooks" in sys.modules:
        return
    mod = types.ModuleType("antenv.axon_hooks")
    _hook = [None]
    mod.set_axon_ntff_profile_hook = lambda h: _hook.__setitem__(0, h)
    mod.get_axon_ntff_profile_hook = lambda: _hook[0]
    import antenv

    antenv.axon_hooks = mod
    sys.modules["antenv.axon_hooks"] = mod
    try:
        sys.path.insert(0, "/root/.axon_site")
        from trn_agent_boot.trn_boot import _ntff_profile_via_ctypes

        mod.set_axon_ntff_profile_hook(
            _ntff_profile_via_ctypes("/opt/axon/libaxon_pjrt.so")
        )
    except Exception as e:  # profiling is best-effort; run still works
        print(f"NTFF hook shim failed ({e}); tracing disabled", file=sys.stderr)


def _run(in_maps, trace=False):
    from concourse.bass_utils import run_bass_kernel_spmd

    nc = _get_program()
    kwargs = {}
    if trace:
        _install_ntff_hook_shim()
        kwargs = {"trace": True, "trace_cores": [0]}
    return run_bass_kernel_spmd(nc, in_maps, core_ids=list(range(N_CORES)), **kwargs)


def kernel(x, W, b):
    res = _run(_shard_inputs(x, W, b))
    return np.concatenate(
        [res.results[c]["out"] for c in range(N_CORES)], axis=0
    ).astype(np.float32)


def kernel_profiled(x, W, b):
    """Like kernel() but also returns the NTFF-measured exec time in ns."""
    res = _run(_shard_inputs(x, W, b), trace=True)
    out = np.concatenate(
        [res.results[c]["out"] for c in range(N_CORES)], axis=0
    ).astype(np.float32)
    return out, res


# revision 9
# speedup vs baseline: 1.0448x; 1.0094x over previous
"""Trainium2 Bass kernel for attention-weighted pooling.

Computes, for x[B,T,D], W[D,1], b[T,1]:
    et = tanh(x @ W + b)            # (B, T)
    at = softmax(et, axis=-1)       # (B, T)
    out = einsum('btd,bt->bd', x, at)

Sharding: pure data parallel over batch across 8 NeuronCores (4 batches per
core); W and b replicated. No collectives.

Key structure (per core, streaming single pass over x):
  - tanh output is bounded in [-1, 1], so softmax needs no max subtraction;
    exp() cannot overflow. Normalization by the denominator is deferred to
    the very end, so x is read from HBM exactly once (memory roofline).
  - x is cast fp32 -> fp16 during the DMA itself (SWDGE cast): HBM traffic
    stays the required 32 MiB/core of fp32, but on-chip x is half the bytes.
  - ALL 16 x chunks (1 MiB fp16 each) stay resident in SBUF so the DMA
    stream is never throttled by buffer reuse.
  - Per 2-MiB chunk [128 x 8 x 512]: ONE whole-chunk DVE tensor_mul
    (fp16 2x mode) + ONE whole-chunk reduce_sum ([P,8,512] -> [P,8,1])
    replace the per-subtile fused dot ops; ACT only does tanh + exp.
  - b is PRE-PERMUTED ON HOST to match the on-chip t-layout, so its DMA is
    one clean [128, 32] HWDGE transfer instead of 2048 8-byte descriptors.
  - Denominator S for every batch via free-dim reduce + ones-matmul on PE
    (no gpsimd custom op -> no mid-stream LOAD_LIB bandwidth dip).
  - Pair-row DMA layout: each descriptor covers 2 adjacent t-rows (4 KiB
    contiguous HBM). Softmax/pooling are permutation-invariant over t.
"""

import sys

sys.path.insert(0, "/opt/trn_rl_repo")

import numpy as np

B, T, D = 32, 4096, 512
N_CORES = 8
B_LOCAL = B // N_CORES          # 4 batches per core
P = 128                         # SBUF partitions
TS_T = 1024                     # t-rows per super-tile (2 MiB fp32 DMA)
N_ST = T // TS_T                # 4 super-tiles per batch
N_J = TS_T // P                 # 8 t-subtiles per super-tile
N_STT = 4                       # subtiles on the fused-DVE path; rest on ACT

# 8-rows-per-partition layout: within a chunk of nj*128 rows starting at t0,
# partition p holds rows t0 + nj*p + j (j = 0..nj-1), so each DMA descriptor
# covers nj*2 KiB contiguous HBM (16 KiB for full chunks -> 4x fewer
# descriptors -> 4x less SWDGE descriptor-ring fetch traffic, which is what
# made SDMA engine 15 a straggler). Softmax/pooling are permutation-invariant
# over t; only the host-side b permutation must match this map.
#
# Batches 0..B_LOCAL-2 stream full 2-MiB chunks; the LAST batch tapers its
# final chunk so the post-stream compute drain is short (the taper's extra
# per-chunk overhead lands after the stream has already ended).
# Plans are (t0, nj, n_stt): chunk covers rows t0 .. t0+nj*128, with n_stt
# dot-products on the fused-DVE path and nj-n_stt on the ACT accum path
# (alternating 5/4 on full chunks balances DVE ~4.8us vs ACT ~4.5us per
# chunk against the ~5.1us DMA pace). Batch 0 tapers its HEAD so compute
# starts ~4us earlier; the last batch tapers its TAIL so the post-stream
# drain is short.
PLAN_HEAD = [(0, 2, 2), (256, 2, 2), (512, 4, 2),
             (1024, 8, 5), (2048, 8, 4), (3072, 8, 5)]
PLAN_FULL = [(0, 8, 4), (1024, 8, 5), (2048, 8, 4), (3072, 8, 5)]
PLAN_TAIL = [(0, 8, 4), (1024, 8, 5), (2048, 8, 4), (3072, 7, 4), (3968, 1, 1)]
PLANS = [PLAN_HEAD, PLAN_FULL, PLAN_FULL, PLAN_TAIL]

_PROGRAM = None


def _build_program():
    import concourse.bacc as bacc
    import concourse.mybir as mybir
    import concourse.tile as tile

    f32 = mybir.dt.float32
    f16 = mybir.dt.float16
    nc = bacc.Bacc("TRN2", target_bir_lowering=False, debug=False)

    x_d = nc.dram_tensor("x", [B_LOCAL, T, D], f32, kind="ExternalInput")
    W_d = nc.dram_tensor("W", [D, 1], f32, kind="ExternalInput")
    b_d = nc.dram_tensor("b", [P, len(PLANS) * N_ST * N_J], f32, kind="ExternalInput")
    o_d = nc.dram_tensor("out", [B_LOCAL, D], f32, kind="ExternalOutput")

    with tile.TileContext(nc) as tc:
        with (
            tc.tile_pool(name="consts", bufs=1) as consts,
            # All 16 x chunks stay resident so the DMA stream is never
            # throttled by SBUF slot reuse.
            tc.tile_pool(name="xin", bufs=16) as xin,
            tc.tile_pool(name="prod", bufs=6) as prod_pool,
            tc.tile_pool(name="small", bufs=8) as small,
            tc.tile_pool(name="pbuf", bufs=2) as pbuf_pool,
            tc.tile_pool(name="acc_psum", bufs=2, space="PSUM") as acc_psum_pool,
            tc.tile_pool(name="s_psum", bufs=2, space="PSUM") as s_psum_pool,
        ):
            plans = PLANS
            xbufs = {8: 14, 7: 1, 4: 1, 2: 2, 1: 1}

            def issue_x_dma(bb, t0, nj):
                # One nj*2 KiB descriptor per partition per chunk.
                xt = xin.tile(
                    [P, nj, D], f16, tag=f"xt{nj}", bufs=xbufs[nj], name="xt"
                )
                nc.gpsimd.dma_start(
                    xt[:],
                    x_d.ap()[bb, t0 : t0 + nj * P, :].rearrange(
                        "(p r) d -> p r d", p=P
                    ),
                )
                return xt

            # Pre-issue the first x chunk so the SWDGE queue starts streaming
            # x before the W broadcast.
            pre = {
                (0, 0): issue_x_dma(0, PLANS[0][0][0], PLANS[0][0][1]),
                (0, 1): issue_x_dma(0, PLANS[0][1][0], PLANS[0][1][1]),
            }

            # W broadcast to all 128 partitions, cast to fp16: [128, D].
            w_bcast = consts.tile([P, D], f16)
            nc.gpsimd.dma_start(
                w_bcast[:],
                W_d.ap().rearrange("d one -> one d").broadcast_to([P, D]),
            )
            # b arrives host-pre-permuted to the on-chip layout(s): section 0
            # for the uniform plan, section 1 for the tapered last batch.
            b_buf = consts.tile([P, len(PLANS) * N_ST * N_J], f32)
            nc.sync.dma_start(b_buf[:], b_d.ap())
            ones_col = consts.tile([P, 1], f32)
            nc.vector.memset(ones_col[:], 1.0)
            # W replicated 4x along the free dim for the merged multiply.
            w_rep = consts.tile([P, 4 * D], f16)
            for j in range(4):
                nc.vector.tensor_copy(w_rep[:, j * D : (j + 1) * D], w_bcast[:])
            # All batch outputs gathered on one partition; single 8 KiB
            # contiguous DMA at the end.
            out_all = consts.tile([1, B_LOCAL * D], f32)

            for bb in range(B_LOCAL):
                p_buf = pbuf_pool.tile([P, T // P], f16)
                acc = acc_psum_pool.tile([1, D], f32)

                chunks = plans[bb]
                sel = bb
                total_mm = N_ST * N_J
                mm_idx = 0
                cum = 0
                for ci, (t0, nj, n_stt) in enumerate(chunks):
                    col0 = cum
                    xt = pre.pop((bb, ci), None)
                    if xt is None:
                        xt = issue_x_dma(bb, t0, nj)
                    # Half the subtiles: fused mult+reduce on DVE
                    # (scalar_tensor_tensor, 1x uop ~690ns). Other half: plain
                    # tensor_tensor mult on DVE (fp16 2x mode ~365ns) with the
                    # reduce offloaded to ACT (activation Copy + accum_out).
                    # This splits the dot-product work so both engines stay
                    # under the DMA pace.
                    # Tiny tail chunks take the all-fused DVE path: one DVE
                    # op has much lower serial latency than mult->ACT->accum.
                    n_act = nj - n_stt
                    elin = small.tile([P, nj], f32, tag=f"elin{nj}")
                    if n_act:
                        # ONE merged tensor_mul for all ACT-path subtiles
                        # (fp16 2x mode; one instruction instead of n_act).
                        prod = prod_pool.tile([P, n_act, D], f16, tag=f"prod{n_act}")
                        nc.vector.tensor_mul(
                            prod[:].rearrange("p j d -> p (j d)"),
                            xt[:, n_stt:nj, :].rearrange("p j d -> p (j d)"),
                            w_rep[:, : n_act * D],
                        )
                    for j in range(n_stt):
                        scratch = prod_pool.tile([P, D], f16, tag="scr")
                        nc.vector.scalar_tensor_tensor(
                            out=scratch[:],
                            in0=xt[:, j, :],
                            scalar=1.0,
                            in1=w_bcast[:],
                            op0=mybir.AluOpType.mult,
                            op1=mybir.AluOpType.mult,
                            accum_out=elin[:, j : j + 1],
                        )
                    for j in range(n_stt, nj):
                        nc.scalar.activation(
                            prod[:, j - n_stt, :],
                            prod[:, j - n_stt, :],
                            mybir.ActivationFunctionType.Copy,
                            accum_out=elin[:, j : j + 1],
                        )
                    ee = small.tile([P, nj], f32, tag=f"ee{nj}")
                    nc.vector.tensor_add(
                        ee[:],
                        elin[:],
                        b_buf[:, sel * 32 + cum : sel * 32 + cum + nj],
                    )
                    et = small.tile([P, nj], f32, tag=f"et{nj}")
                    nc.scalar.activation(
                        et[:], ee[:], mybir.ActivationFunctionType.Tanh
                    )
                    nc.scalar.activation(
                        p_buf[:, col0 : col0 + nj],
                        et[:],
                        mybir.ActivationFunctionType.Exp,
                    )
                    cum += nj
                    for j in range(nj):
                        nc.tensor.matmul(
                            acc[:],
                            p_buf[:, col0 + j : col0 + j + 1],
                            xt[:, j, :],
                            start=(mm_idx == 0),
                            stop=(mm_idx == total_mm - 1),
                        )
                        mm_idx += 1

                # denominator S = sum_t p_t  (free-dim reduce, then
                # cross-partition reduce via ones-matmul on PE; the tiny S
                # matmul queues right behind the batch's pooling matmuls)
                ssum = small.tile([P, 1], f32)
                nc.vector.reduce_sum(ssum[:], p_buf[:], axis=mybir.AxisListType.X)
                s_ps = s_psum_pool.tile([1, 1], f32)
                nc.tensor.matmul(s_ps[:], ssum[:], ones_col[:])
                sinv = small.tile([1, 1], f32)
                nc.vector.reciprocal(sinv[:], s_ps[:])
                if bb < B_LOCAL - 1:
                    # Mid-stream: final scale on ACT (it is idle), overlapping
                    # the ongoing stream.
                    nc.scalar.mul(
                        out_all[:, bb * D : (bb + 1) * D], acc[:], sinv[:]
                    )
                else:
                    # Last batch: scale on DVE right after its own reciprocal
                    # (no cross-engine hop on the exposed tail).
                    nc.vector.tensor_scalar_mul(
                        out_all[:, bb * D : (bb + 1) * D], acc[:], sinv[:]
                    )

            nc.sync.dma_start(
                o_d.ap().rearrange("(one b) d -> one (b d)", one=1), out_all[:]
            )

    nc.compile()
    return nc


def _get_program():
    global _PROGRAM
    if _PROGRAM is None:
        _PROGRAM = _build_program()
    return _PROGRAM


def _permute_b(b):
    """Host-side: lay b out as [P, 32*len(PLANS)] matching the per-batch
    on-chip layouts."""
    bv = b[:, 0]
    out = np.zeros((P, len(PLANS) * N_ST * N_J), dtype=np.float32)
    p = np.arange(P)
    for sec, plan in enumerate(PLANS):
        cum = 0
        for t0, nj, _ in plan:
            for j in range(nj):
                out[:, sec * 32 + cum + j] = bv[t0 + nj * p + j]
            cum += nj
    return np.ascontiguousarray(out)


def _shard_inputs(x, W, b):
    x = np.ascontiguousarray(np.asarray(x, dtype=np.float32))
    W = np.ascontiguousarray(np.asarray(W, dtype=np.float32))
    b = np.ascontiguousarray(np.asarray(b, dtype=np.float32))
    b_perm = _permute_b(b)
    return [
        {"x": x[c * B_LOCAL : (c + 1) * B_LOCAL], "W": W, "b": b_perm}
        for c in range(N_CORES)
    ]


def _install_ntff_hook_shim():
    """The agent image's ``antenv`` lacks ``axon_hooks``, so the boot-time
    NTFF hook registration silently degrades. Recreate the module in
    sys.modules and register the ctypes hook against libaxon_pjrt.so."""
    import types

    if "antenv.axon_hooks" in sys.modules:
        return
    mod = types.ModuleType("antenv.axon_hooks")
    _hook = [None]
    mod.set_axon_ntff_profile_hook = lambda h: _hook.__setitem__(0, h)
    mod.get_axon_ntff_profile_hook = lambda: _hook[0]
    import antenv

    antenv.axon_hooks = mod
    sys.modules["antenv.axon_hooks"] = mod
    try:
        sys.path.insert(0, "/root/.axon_site")
        from trn_agent_boot.trn_boot import _ntff_profile_via_ctypes

        mod.set_axon_ntff_profile_hook(
            _ntff_profile_via_ctypes("/opt/axon/libaxon_pjrt.so")
        )
    except Exception as e:  # profiling is best-effort; run still works
        print(f"NTFF hook shim failed ({e}); tracing disabled", file=sys.stderr)


def _run(in_maps, trace=False):
    from concourse.bass_utils import run_bass_kernel_spmd

    nc = _get_program()
    kwargs = {}
    if trace:
        _install_ntff_hook_shim()
        kwargs = {"trace": True, "trace_cores": [0]}
    return run_bass_kernel_spmd(nc, in_maps, core_ids=list(range(N_CORES)), **kwargs)


def kernel(x, W, b):
    res = _run(_shard_inputs(x, W, b))
    return np.concatenate(
        [res.results[c]["out"] for c in range(N_CORES)], axis=0
    ).astype(np.float32)


def kernel_profiled(x, W, b):
    """Like kernel() but also returns the NTFF-measured exec time in ns."""
    res = _run(_shard_inputs(x, W, b), trace=True)
    out = np.concatenate(
        [res.results[c]["out"] for c in range(N_CORES)], axis=0
    ).astype(np.float32)
    return out, res
